# revision 42
# baseline (speedup 1.0000x reference)
"""Trainium2 Bass kernel for nn_CannyDetector (B=8, Cin=3, C=8, H=W=512).

Strategy (pure data parallel, one image per NeuronCore, 8 cores):

Reference pipeline per core:
    h  = Wexp @ x; g = Pg @ gauss3x3(h); gx = Px @ sobelx(g); gy = Py @ sobely(g)
    mag = sqrt(gx^2+gy^2+eps); s = sum_c mag; dirs_k = s - shift_k(s)
    idx = round(atan2(gy,gx)/45deg); nms = mag * (dirs[idx]>0 & dirs[idx+4]>0)
    thr = step functions of mag; m = Wmerge @ thr; out = hysteresis(m)

This implementation folds ALL linear work (gauss/sobel separable 5x5
stencils + channel mixes + the +22.5deg rotation) into TensorE:
  - one DVE op per stripe builds the c2 = x[w-1]+x[w+1] helper slots; the
    moving operand {x, c2} (120 partitions = 2 slots x 3ch x 20 rows) then
    needs only 3 column-shifted accumulating matmuls per component.
  - components are p1,p2 (gradients rotated by +22.5deg).  The NMS
    sector tests reduce to sign tests: beta = [p1^2>=p2^2],
    alpha = [2*q1^2>=mag2] == [p1*p2>=0] == [sign(p1)==sign(p2)].
  - ScalarE emits squares and signs of the PE outputs (PSUM->SBUF bf16);
    the remaining per-channel chain is 7 TT + 2 TS DVE ops per
    half-stripe, software-pipelined so PE pass-1(s+1) overlaps chain(s).
  - PE pass 2 contracts channels into 4 axis-plane tensors C_k and
    s = sum_c mag with an interleaved column layout (col = 5*row + plane)
    so ONE SBUF->SBUF DMA per chunk repacks all 5 planes into the
    full-image row-major tail layout.
  - the 1-channel tail (directional NMS gating + hysteresis) runs
    per 128-row block, pipelined into the stripe loop, so only the last
    block's work trails the final stripe.
"""

import math
import sys

import numpy as np

if "/opt/trn_rl_repo" not in sys.path:
    sys.path.insert(0, "/opt/trn_rl_repo")

import ml_dtypes

import concourse.bass as bass
import concourse.bacc as bacc
import concourse.tile as tile
from concourse import mybir
from concourse.bass_utils import run_bass_kernel_spmd

F32 = mybir.dt.float32
BF16 = mybir.dt.bfloat16
OP = mybir.AluOpType
AF = mybir.ActivationFunctionType

EPS = 1e-10
NEG_BIG = -1.0e30

R = 16           # rows per chunk (PE pass output = 8ch x 16 rows)
KROWS = R + 4    # vertical halo rows per chunk (5-tap)
SCH = 4          # chunks per stripe
DEBUG = False    # adds a dbg m-plane output when building the program


def make_weights(w_expand, w_perm_gauss, w_perm_sx, w_perm_sy, w_merge,
                 low_t, high_t, H):
    """Host-side prep of stationary matrices and per-partition vectors."""
    e = math.exp(-1.0 / 18.0)
    n = 1.0 / (2.0 * e + 1.0)

    Ax = (w_perm_sx @ w_perm_gauss @ w_expand).astype(np.float64)   # (8,3)
    Ay = (w_perm_sy @ w_perm_gauss @ w_expand).astype(np.float64)

    gv = np.array([e, 1.0, e]) * n
    VX = np.convolve(np.array([0.5, 1.0, 0.5]), gv)   # qx vertical 5-tap
    VY = np.convolve(np.array([-1.0, 0.0, 1.0]), gv)  # qy vertical 5-tap

    # horizontal FIRs decomposed over moving slots {x, c2=x[w-1]+x[w+1]}
    # qx path: H = conv([-1,0,1], [e,1,e])*n ; qy path: H = conv([.5,1,.5],[e,1,e])*n
    axx = {-1: -n, 0: 0.0, 1: n}            # x slot, qx
    axc = {-1: -e * n, 0: 0.0, 1: e * n}    # c2 slot, qx
    ayx = {-1: (0.5 + e) * n, 0: n, 1: (0.5 + e) * n}  # x slot, qy
    ayc = {-1: 0.5 * e * n, 0: 0.0, 1: 0.5 * e * n}    # c2 slot, qy

    cth, sth = math.cos(math.pi / 8), math.sin(math.pi / 8)
    comps = [
        (cth, -sth),               # p1
        (sth, cth),                # p2
    ]

    CH = H // R
    pos_chunk = {0: 0, 1: min(1, CH - 1), 2: CH - 1}
    # stationaries [18, 124, 128]: x slots rows 0..59, c2 slots rows 64..123
    # (rows 60..63 are zero padding -- DVE writes must start at partition
    # 0/32/64/96, so the c2 block sits at 64); index ((ci*3 + pos)*3 + o)
    wst = np.zeros((18, 124, 128), np.float64)
    for ci, (wx, wy) in enumerate(comps):
        for pos in range(3):
            c0 = pos_chunk[pos]
            start = min(max(R * c0 - 2, 0), H - KROWS)
            for o in range(3):                 # column shift o-1 in {-1,0,1}
                Wm = wst[(ci * 3 + pos) * 3 + o]
                cx = wx * axx[o - 1]
                cxc = wx * axc[o - 1]
                cy = wy * ayx[o - 1]
                cyc = wy * ayc[o - 1]
                for o8 in range(8):
                    for r in range(R):
                        y = R * c0 + r
                        col = R * o8 + r
                        for k in range(-2, 3):
                            v = y + k
                            if v < 0:
                                v = -v
                            if v > H - 1:
                                v = 2 * (H - 1) - v
                            rp = v - start
                            assert 0 <= rp < KROWS
                            for cc in range(3):
                                wxa = Ax[o8, cc] * VX[k + 2]
                                wya = Ay[o8, cc] * VY[k + 2]
                                Wm[cc * KROWS + rp, col] += cx * wxa + cy * wya
                                Wm[64 + cc * KROWS + rp, col] += cxc * wxa + cyc * wya

    # pass-2 stationaries [5, 128, 80]: col = 5*j + plane  (j = row in chunk)
    wp = (w_merge[0].astype(np.float64)) * 0.5
    lam = {
        0: np.array([0.0, 0.0, 1.0, 0.0]),    # g   -> C2
        1: np.array([0.0, 1.0, -1.0, 0.0]),   # gA  -> C1 - C2
        2: np.array([0.0, 0.0, -1.0, 1.0]),   # gB  -> C3 - C2
        3: np.array([1.0, -1.0, 1.0, -1.0]),  # gAB -> C0 - C1 + C2 - C3
    }
    wc = np.zeros((5, 128, 80), np.float64)
    for T in range(4):
        for o8 in range(8):
            for j in range(R):
                for k in range(4):
                    wc[T, R * o8 + j, 5 * j + k] = wp[o8] * lam[T][k]
    for o8 in range(8):
        for j in range(R):
            wc[4, R * o8 + j, 5 * j + 4] = 1.0

    lt = low_t.reshape(-1).astype(np.float64)
    ht = high_t.reshape(-1).astype(np.float64)
    l2v = np.zeros((128, 1), np.float32)
    h2v = np.zeros((128, 1), np.float32)
    for o8 in range(8):
        l2v[R * o8:R * (o8 + 1), 0] = (lt[o8] * lt[o8]) if lt[o8] >= 0 else NEG_BIG
        h2v[R * o8:R * (o8 + 1), 0] = (ht[o8] * ht[o8]) if ht[o8] >= 0 else NEG_BIG

    return {
        # stored pre-transposed to the SBUF tile layouts for contiguous DMAs
        "wst": np.ascontiguousarray(wst.transpose(1, 0, 2)).astype(ml_dtypes.bfloat16),
        "wc": np.ascontiguousarray(wc.transpose(1, 0, 2)).astype(ml_dtypes.bfloat16),
        "l2v": l2v,
        "h2v": h2v,
    }


def chunk_start(c, H):
    return min(max(R * c - 2, 0), H - KROWS)


def chunk_pos(c, H):
    CH = H // R
    return 0 if c == 0 else (2 if c == CH - 1 else 1)


def build_program(H, W, n_cores, debug=False):
    """Emit the single-core Tile program (SPMD across cores)."""
    Wp = W + 5               # padded x columns: x[-3 .. W+1]
    CH = H // R
    NS = H // (R * SCH)
    PB = 128
    HB = H // PB
    CPB = PB // R            # chunks per block
    assert H % (R * SCH) == 0 and PB % R == 0 and (SCH * 2) * R == PB

    nc = bacc.Bacc("TRN2", target_bir_lowering=False, debug=False,
                   enable_asserts=True, num_devices=n_cores)

    # x pre-windowed on host: [NS, 60, SCH, Wp] flattened (+8 slack so the
    # +1-shifted load stays in bounds) -> ONE fully-contiguous DMA per
    # stripe for xcm and one for xc2.
    XWN = NS * 60 * SCH * Wp
    xin_t = nc.dram_tensor("xin", (XWN + 8,), BF16, kind="ExternalInput")
    wst_t = nc.dram_tensor("wst", (124, 18, 128), BF16, kind="ExternalInput")
    wc_t = nc.dram_tensor("wc", (128, 5, 80), BF16, kind="ExternalInput")
    l2v_t = nc.dram_tensor("l2v", (128, 1), F32, kind="ExternalInput")
    h2v_t = nc.dram_tensor("h2v", (128, 1), F32, kind="ExternalInput")
    out_t = nc.dram_tensor("out", (H, W), F32, kind="ExternalOutput")
    dbg_t = None
    if debug:
        dbg_t = nc.dram_tensor("dbg", (H, W), F32, kind="ExternalOutput")

    def dram_ap(t, offset, pairs):
        return bass.AP(t, offset, [list(p) for p in pairs])

    from contextlib import ExitStack
    with tile.TileContext(nc) as tc, ExitStack() as stack:
        cpool = stack.enter_context(tc.tile_pool(name="consts", bufs=1))
        wstt = cpool.tile([124, 18, 128], BF16, tag="wst", name="wst")
        nc.scalar.dma_start(wstt[:], wst_t.ap())
        wst = [wstt[:, i, :] for i in range(18)]
        wct = cpool.tile([128, 5, 80], BF16, tag="wc", name="wc")
        nc.scalar.dma_start(wct[:], wc_t.ap())
        wc = [wct[:, i, :] for i in range(5)]
        l2v = cpool.tile([128, 1], F32, tag="l2v", name="l2v")
        nc.scalar.dma_start(l2v[:], l2v_t.ap())
        h2v = cpool.tile([128, 1], F32, tag="h2v", name="h2v")
        nc.scalar.dma_start(h2v[:], h2v_t.ap())
        epsv = cpool.tile([128, 1], F32, tag="epsv", name="epsv")
        nc.gpsimd.memset(epsv[:], float(EPS))

        # full-image planes: P5 holds [C0..C3, s] interleaved per block.
        ppool = stack.enter_context(tc.tile_pool(name="planes", bufs=1))
        P5 = ppool.tile([PB, 5, HB, W], BF16, tag="P5", name="P5")
        spl = ppool.tile([PB, HB, W + 2], BF16, tag="spl", name="spl")
        sSp = ppool.tile([PB, HB, W + 2], BF16, tag="sSp", name="sSp")
        sNp = ppool.tile([PB, HB, W + 2], BF16, tag="sNp", name="sNp")
        mpl = ppool.tile([PB, HB, W], F32, tag="mpl", name="mpl")
        hhpl = ppool.tile([PB, HB, W], BF16, tag="hhpl", name="hhpl")
        iScpl = ppool.tile([PB, HB, W], BF16, tag="iScpl", name="iScpl")

        # zero-fill once: pad cols + out-of-image halos stay zero (the dirs
        # conv uses zero padding); block DMAs overwrite the interiors.
        for t in (spl, sSp, sNp):
            nc.gpsimd.memset(t[:], 0.0)

        with (
            tc.tile_pool(name="xs", bufs=2) as xpool,
            tc.tile_pool(name="sq", bufs=3) as spool,
            tc.tile_pool(name="dv", bufs=2) as dpool,
            tc.tile_pool(name="s2", bufs=3) as s2pool,
            tc.tile_pool(name="tl", bufs=1) as tpool,
            tc.tile_pool(name="ps", bufs=2, space="PSUM") as mm1pool,
            tc.tile_pool(name="ps2", bufs=2, space="PSUM") as mm2pool,
        ):
            stripe_state = {}

            def emit_A(s):
                """Loads + c2 + PE pass 1 + squares/signs."""
                cbase = s * SCH
                xcm = xpool.tile([124, SCH, Wp], BF16, tag="xcm", name="xcm")
                xc2 = xpool.tile([60, SCH, Wp], BF16, tag="xc2", name="xc2")
                # pad rows 60..63: any finite data (stationary rows are zero)
                nc.gpsimd.dma_start(
                    xcm[60:64, :, :],
                    dram_ap(xin_t, 0, [(Wp, 4), (Wp, SCH), (1, Wp)]))
                base = s * (60 * SCH * Wp)
                nc.sync.dma_start(
                    xcm[0:60, :, :],
                    dram_ap(xin_t, base, [(SCH * Wp, 60), (Wp, SCH), (1, Wp)]))
                nc.sync.dma_start(
                    xc2[:, :, :],
                    dram_ap(xin_t, base + 1, [(SCH * Wp, 60), (Wp, SCH), (1, Wp)]))
                # c2 slots: c2[t] = x[t-4] + x[t-2] (both from the +1-shifted copy)
                nc.vector.tensor_tensor(
                    xcm[64:124, :, 2:Wp - 1], xc2[:, :, 0:Wp - 3],
                    xc2[:, :, 2:Wp - 1], OP.add)

                # PE pass 1 (p1, p2): one PSUM tile per chunk; Square + Sign ACTs
                SPall = spool.tile([128, SCH, 2, W], BF16, tag="spall", name="spall")
                SGall = spool.tile([128, SCH, 2, W], BF16, tag="sgall", name="sgall")
                for cl in range(SCH):
                    pos = chunk_pos(cbase + cl, H)
                    pm = mm1pool.tile([128, 2, W], F32, tag="pm", name="pm")
                    for ci in range(2):
                        for o in range(3):
                            nc.tensor.matmul(
                                pm[:, ci, :], wst[(ci * 3 + pos) * 3 + o],
                                xcm[0:124, cl, 2 + o:2 + o + W],
                                start=(o == 0), stop=(o == 2))
                    nc.scalar.activation(SPall[:, cl, :, :], pm[:], AF.Square)
                    nc.scalar.activation(SGall[:, cl, :, :], pm[:], AF.Sign)
                stripe_state[s] = {"SPall": SPall, "SGall": SGall}

            def emit_B(s, h):
                """DVE chain + sqrt for half-stripe h (chunks 2h, 2h+1)."""
                st = stripe_state[s]
                sl = slice(2 * h, 2 * h + 2)
                sp1 = st["SPall"][:, sl, 0, :]
                sp2 = st["SPall"][:, sl, 1, :]
                mag2 = dpool.tile([128, 2, W], BF16, tag="mag2", name="mag2")
                nc.vector.tensor_tensor(mag2[:], sp1, sp2, OP.add)
                bm = dpool.tile([128, 2, W], BF16, tag="bm", name="bm")
                nc.vector.tensor_tensor(bm[:], sp1, sp2, OP.is_ge)
                # alpha = [p1*p2 >= 0] == [sign(p1) == sign(p2)] (ties measure-zero)
                am = dpool.tile([128, 2, W], BF16, tag="am", name="am")
                nc.vector.tensor_tensor(am[:], st["SGall"][:, sl, 0, :],
                                        st["SGall"][:, sl, 1, :], OP.is_equal)
                stepl = dpool.tile([128, 2, W], BF16, tag="stepl", name="stepl")
                nc.vector.tensor_scalar(stepl[:], mag2[:], l2v[:], None, OP.is_ge)
                steph = dpool.tile([128, 2, W], BF16, tag="steph", name="steph")
                nc.vector.tensor_scalar(steph[:], mag2[:], h2v[:], None, OP.is_ge)
                g = dpool.tile([128, 2, W], BF16, tag="g", name="g")
                nc.vector.tensor_tensor(g[:], stepl[:], steph[:], OP.add)
                gA = dpool.tile([128, 2, W], BF16, tag="gA", name="gA")
                nc.vector.tensor_tensor(gA[:], am[:], g[:], OP.mult)
                gB = dpool.tile([128, 2, W], BF16, tag="gB", name="gB")
                nc.vector.tensor_tensor(gB[:], bm[:], g[:], OP.mult)
                gAB = dpool.tile([128, 2, W], BF16, tag="gAB", name="gAB")
                nc.vector.tensor_tensor(gAB[:], am[:], gB[:], OP.mult)
                mag = dpool.tile([128, 2, W], BF16, tag="mag", name="mag")
                nc.scalar.activation(mag[:], mag2[:], AF.Sqrt, bias=epsv[:])
                st[("half", h)] = (g, gA, gB, gAB, mag)

            def emit_C(s, h):
                """PE pass 2 + PSUM copy + repack for half-stripe h."""
                st = stripe_state[s]
                g, gA, gB, gAB, mag = st.pop(("half", h))
                gT = (g, gA, gB, gAB)
                pc = mm2pool.tile([80, 2, W], F32, tag="pc", name="pc")
                for cl2h in range(2):
                    for T in range(4):
                        nc.tensor.matmul(pc[:, cl2h, :], wc[T], gT[T][:, cl2h, :],
                                         start=(T == 0), stop=False)
                    nc.tensor.matmul(pc[:, cl2h, :], wc[4], mag[:, cl2h, :],
                                     start=False, stop=True)
                stC = s2pool.tile([80, 2, W], BF16, tag="stC", name="stC")
                nc.scalar.activation(stC[:], pc[:], AF.Copy)
                for cl2h in range(2):
                    c = s * SCH + 2 * h + cl2h
                    b = c // CPB
                    cl2 = c % CPB
                    nc.gpsimd.dma_start(P5[R * cl2:R * (cl2 + 1), :, b, :],
                                        stC[:, cl2h, :])

            def emit_nms_planes(b, p0=0, p1=PB):
                """Shifted s-plane DMAs for block b (interior + top halo).
                All on HWDGE rings: gpsimd SWDGE descriptor generation is
                port-locked out while DVE runs 2-port ops."""
                nc.scalar.dma_start(spl[p0:p1, b, 1:W + 1], P5[p0:p1, 4, b, :])
                # sSp[p] = s[row-1] (N side), sNp[p] = s[row+1] (S side)
                lo = max(p0, 1)
                nc.sync.dma_start(sSp[lo:p1, b, 1:W + 1], P5[lo - 1:p1 - 1, 4, b, :])
                if p0 == 0 and b > 0:
                    nc.sync.dma_start(sSp[0:1, b, 1:W + 1], P5[PB - 1:PB, 4, b - 1, :])
                hi = min(p1, PB - 1)
                nc.sync.dma_start(sNp[p0:hi, b, 1:W + 1], P5[p0 + 1:hi + 1, 4, b, :])

            def emit_nms(b, p0=0, p1=PB, planes_done=False):
                """NMS gating for block b, partition rows [p0, p1)."""
                n = p1 - p0
                sc = P5[p0:p1, 4, b, :]
                if not planes_done:
                    emit_nms_planes(b, p0, p1)
                if p1 == PB and b < HB - 1:
                    nc.sync.dma_start(sNp[PB - 1:PB, b, 1:W + 1], P5[0:1, 4, b + 1, :])

                # axis planes: C0=E/W, C1=SE/NW, C2=S/N, C3=SW/NE
                pairs = [
                    (spl[p0:p1, b, 2:W + 2], spl[p0:p1, b, 0:W]),      # E, W
                    (sNp[p0:p1, b, 2:W + 2], sSp[p0:p1, b, 0:W]),      # SE, NW
                    (sSp[p0:p1, b, 1:W + 1], sNp[p0:p1, b, 1:W + 1]),  # N, S
                    (sNp[p0:p1, b, 0:W], sSp[p0:p1, b, 2:W + 2]),      # SW, NE
                ]
                pks = []
                for k, (va, vb) in enumerate(pairs):
                    mx = tpool.tile([PB, W], BF16, tag="mx", name="mx")
                    nc.vector.tensor_tensor(mx[p0:p1, :], va, vb, OP.max)
                    bk = tpool.tile([PB, W], BF16, tag="bk", name="bk")
                    nc.vector.tensor_tensor(bk[p0:p1, :], sc, mx[p0:p1, :], OP.is_gt)
                    pk = tpool.tile([PB, W], BF16, tag=f"pk{k}", name=f"pk{k}")
                    nc.vector.tensor_tensor(pk[p0:p1, :], bk[p0:p1, :],
                                            P5[p0:p1, k, b, :], OP.mult)
                    pks.append(pk)
                a01 = tpool.tile([PB, W], F32, tag="a01", name="a01")
                nc.vector.tensor_tensor(a01[p0:p1, :], pks[0][p0:p1, :],
                                        pks[1][p0:p1, :], OP.add)
                a23 = tpool.tile([PB, W], F32, tag="a23", name="a23")
                nc.vector.tensor_tensor(a23[p0:p1, :], pks[2][p0:p1, :],
                                        pks[3][p0:p1, :], OP.add)
                nc.vector.tensor_tensor(mpl[p0:p1, b, :], a01[p0:p1, :],
                                        a23[p0:p1, :], OP.add)

                # strong map + horizontal part of the hysteresis box sum
                iSp = tpool.tile([PB, W + 2], BF16, tag="iSp", name="iSp")
                nc.vector.tensor_scalar(iSp[p0:p1, 1:W + 1], mpl[p0:p1, b, :],
                                        1.0, None, OP.is_equal)
                nc.vector.tensor_scalar(iScpl[p0:p1, b, :], mpl[p0:p1, b, :],
                                        1.0, None, OP.is_equal)
                # reflect pad: iS[-1] = iS[1], iS[W] = iS[W-2]
                nc.vector.tensor_copy(iSp[p0:p1, 0:1], iSp[p0:p1, 2:3])
                nc.vector.tensor_copy(iSp[p0:p1, W + 1:W + 2], iSp[p0:p1, W - 1:W])
                th = tpool.tile([PB, W], BF16, tag="th", name="th")
                nc.vector.tensor_tensor(th[p0:p1, :], iSp[p0:p1, 0:W],
                                        iSp[p0:p1, 2:W + 2], OP.add)
                nc.vector.tensor_tensor(hhpl[p0:p1, b, :], th[p0:p1, :],
                                        iScpl[p0:p1, b, :], OP.add)
                if dbg_t is not None:
                    nc.sync.dma_start(
                        dram_ap(dbg_t, b * PB * W + p0 * W, [(W, n), (1, W)]),
                        mpl[p0:p1, b, :])

            def emit_hyst(b):
                hhN = tpool.tile([PB, W], BF16, tag="hhN", name="hhN")
                hhS = tpool.tile([PB, W], BF16, tag="hhS", name="hhS")
                nc.sync.dma_start(hhN[0:PB - 1, :], hhpl[1:PB, b, :])
                nc.scalar.dma_start(hhS[1:PB, :], hhpl[0:PB - 1, b, :])
                if b < HB - 1:
                    nc.sync.dma_start(hhN[PB - 1:PB, :], hhpl[0:1, b + 1, :])
                else:  # reflect at y = H-1
                    nc.sync.dma_start(hhN[PB - 1:PB, :], hhpl[PB - 2:PB - 1, b, :])
                if b > 0:
                    nc.scalar.dma_start(hhS[0:1, :], hhpl[PB - 1:PB, b - 1, :])
                else:  # reflect at y = 0
                    nc.scalar.dma_start(hhS[0:1, :], hhpl[1:2, b, :])
                vv = tpool.tile([PB, W], BF16, tag="vv", name="vv")
                nc.vector.tensor_tensor(vv[:], hhN[:], hhS[:], OP.add)
                hv = tpool.tile([PB, W], BF16, tag="hv", name="hv")
                nc.vector.tensor_tensor(hv[:], vv[:], hhpl[:, b, :], OP.add)
                # w2 = 0.5*[hv > 0.8]; w1 = w2*[m == 0.5]; out = w1 + iS
                w2 = tpool.tile([PB, W], BF16, tag="w2", name="w2")
                nc.vector.tensor_scalar(w2[:], hv[:], 0.8, 0.5, OP.is_gt, OP.mult)
                m5 = tpool.tile([PB, W], BF16, tag="m5", name="m5")
                nc.vector.tensor_scalar(m5[:], mpl[:, b, :], 0.5, None, OP.is_equal)
                w1 = tpool.tile([PB, W], BF16, tag="w1", name="w1")
                nc.vector.tensor_tensor(w1[:], w2[:], m5[:], OP.mult)
                outb = tpool.tile([PB, W], BF16, tag="outb", name="outb")
                nc.vector.tensor_tensor(outb[:], w1[:], iScpl[:, b, :], OP.add)
                outv = tpool.tile([PB, W], F32, tag="outv", name="outv")
                nc.scalar.activation(outv[:], outb[:], AF.Copy)
                nc.sync.dma_start(
                    dram_ap(out_t, b * PB * W, [(W, PB), (1, W)]), outv[:])

            emit_A(0)
            for s in range(NS):
                if s + 1 < NS:
                    emit_A(s + 1)
                last = (s == NS - 1)
                for h in range(2):
                    emit_B(s, h)
                    emit_C(s, h)
                    if last:
                        # last block split by partition halves so the
                        # endgame latency chain is short
                        emit_nms(HB - 1, 64 * h, 64 * (h + 1))
                        if h == 0:
                            emit_hyst(HB - 2)
                stripe_state.pop(s)
                if not last and s % 2 == 1:
                    # interior shifted-plane DMAs can start one stripe early
                    emit_nms_planes((s - 1) // 2)
                if not last and s >= 2 and s % 2 == 0:
                    emit_nms(s // 2 - 1, planes_done=True)
                if not last and s >= 4 and s % 2 == 0:
                    emit_hyst(s // 2 - 2)
            emit_hyst(HB - 1)

    nc.compile()
    return nc


_PROG_CACHE = {}


def _get_program(H, W, n_cores):
    key = (H, W, n_cores, DEBUG)
    if key not in _PROG_CACHE:
        _PROG_CACHE[key] = build_program(H, W, n_cores, debug=DEBUG)
    return _PROG_CACHE[key]


def make_in_maps(x, w_expand, w_perm_gauss, w_perm_sx, w_perm_sy, w_merge,
                 low_t, high_t):
    B, Cin, H, W = x.shape
    Wp = W + 5
    NS = H // (R * SCH)
    wd = make_weights(np.asarray(w_expand, np.float64), np.asarray(w_perm_gauss, np.float64),
                      np.asarray(w_perm_sx, np.float64), np.asarray(w_perm_sy, np.float64),
                      np.asarray(w_merge, np.float64), np.asarray(low_t), np.asarray(high_t), H)
    xpad = np.pad(np.asarray(x, np.float32), ((0, 0), (0, 0), (0, 0), (3, 2)),
                  mode="reflect").astype(ml_dtypes.bfloat16)
    # pre-windowed layout [NS, 60 = 3ch x KROWS, SCH, Wp] (+8 slack elements)
    xwin = np.zeros((B, NS, 3, KROWS, SCH, Wp), ml_dtypes.bfloat16)
    for s in range(NS):
        for cl in range(SCH):
            st = chunk_start(s * SCH + cl, H)
            xwin[:, s, :, :, cl, :] = xpad[:, :, st:st + KROWS, :]
    xflat = np.zeros((B, NS * 60 * SCH * Wp + 8), ml_dtypes.bfloat16)
    xflat[:, :-8] = xwin.reshape(B, -1)
    shared = {
        "wst": np.ascontiguousarray(wd["wst"]),
        "wc": np.ascontiguousarray(wd["wc"]),
        "l2v": wd["l2v"], "h2v": wd["h2v"],
    }
    return [dict(shared, xin=xflat[b]) for b in range(B)]


def kernel(x, w_expand, w_perm_gauss, w_perm_sx, w_perm_sy, w_merge,
           low_t, high_t):
    x = np.asarray(x)
    B, Cin, H, W = x.shape
    assert (B, Cin) == (8, 3)
    nc = _get_program(H, W, 8)
    in_maps = make_in_maps(x, w_expand, w_perm_gauss, w_perm_sx, w_perm_sy,
                           w_merge, low_t, high_t)
    res = run_bass_kernel_spmd(nc, in_maps, core_ids=list(range(8)))
    out = np.stack([res.results[b]["out"] for b in range(8)])[:, None]
    return out.astype(np.float32)


# revision 46
# speedup vs baseline: 1.0556x; 1.0556x over previous
"""Trainium2 Bass kernel for nn_CannyDetector (B=8, Cin=3, C=8, H=W=512).

Strategy (pure data parallel, one image per NeuronCore, 8 cores):

Reference pipeline per core:
    h  = Wexp @ x; g = Pg @ gauss3x3(h); gx = Px @ sobelx(g); gy = Py @ sobely(g)
    mag = sqrt(gx^2+gy^2+eps); s = sum_c mag; dirs_k = s - shift_k(s)
    idx = round(atan2(gy,gx)/45deg); nms = mag * (dirs[idx]>0 & dirs[idx+4]>0)
    thr = step functions of mag; m = Wmerge @ thr; out = hysteresis(m)

This implementation folds ALL linear work (gauss/sobel separable 5x5
stencils + channel mixes + the +22.5deg rotation) into TensorE:
  - one DVE op per stripe builds the c2 = x[w-1]+x[w+1] helper slots; the
    moving operand {x, c2} (120 partitions = 2 slots x 3ch x 20 rows) then
    needs only 3 column-shifted accumulating matmuls per component.
  - components are p1,p2 (gradients rotated by +22.5deg).  The NMS
    sector tests reduce to sign tests: beta = [p1^2>=p2^2],
    alpha = [2*q1^2>=mag2] == [p1*p2>=0] == [sign(p1)==sign(p2)].
  - ScalarE emits squares and signs of the PE outputs (PSUM->SBUF bf16);
    the remaining per-channel chain is 7 TT + 2 TS DVE ops per
    half-stripe, software-pipelined so PE pass-1(s+1) overlaps chain(s).
  - PE pass 2 contracts channels into 4 axis-plane tensors C_k and
    s = sum_c mag with an interleaved column layout (col = 5*row + plane)
    so ONE SBUF->SBUF DMA per chunk repacks all 5 planes into the
    full-image row-major tail layout.
  - the 1-channel tail (directional NMS gating + hysteresis) runs
    per 128-row block, pipelined into the stripe loop, so only the last
    block's work trails the final stripe.
"""

import math
import sys

import numpy as np

if "/opt/trn_rl_repo" not in sys.path:
    sys.path.insert(0, "/opt/trn_rl_repo")

import ml_dtypes

import concourse.bass as bass
import concourse.bacc as bacc
import concourse.tile as tile
from concourse import mybir
from concourse.bass_utils import run_bass_kernel_spmd

F32 = mybir.dt.float32
BF16 = mybir.dt.bfloat16
OP = mybir.AluOpType
AF = mybir.ActivationFunctionType

EPS = 1e-10
NEG_BIG = -1.0e30

R = 16           # rows per chunk (PE pass output = 8ch x 16 rows)
KROWS = R + 4    # vertical halo rows per chunk (5-tap)
SCH = 4          # chunks per stripe
DEBUG = False    # adds a dbg m-plane output when building the program


def make_weights(w_expand, w_perm_gauss, w_perm_sx, w_perm_sy, w_merge,
                 low_t, high_t, H):
    """Host-side prep of stationary matrices and per-partition vectors."""
    e = math.exp(-1.0 / 18.0)
    n = 1.0 / (2.0 * e + 1.0)

    Ax = (w_perm_sx @ w_perm_gauss @ w_expand).astype(np.float64)   # (8,3)
    Ay = (w_perm_sy @ w_perm_gauss @ w_expand).astype(np.float64)

    gv = np.array([e, 1.0, e]) * n
    VX = np.convolve(np.array([0.5, 1.0, 0.5]), gv)   # qx vertical 5-tap
    VY = np.convolve(np.array([-1.0, 0.0, 1.0]), gv)  # qy vertical 5-tap

    # horizontal FIRs decomposed over moving slots {x, c2=x[w-1]+x[w+1]}
    # qx path: H = conv([-1,0,1], [e,1,e])*n ; qy path: H = conv([.5,1,.5],[e,1,e])*n
    axx = {-1: -n, 0: 0.0, 1: n}            # x slot, qx
    axc = {-1: -e * n, 0: 0.0, 1: e * n}    # c2 slot, qx
    ayx = {-1: (0.5 + e) * n, 0: n, 1: (0.5 + e) * n}  # x slot, qy
    ayc = {-1: 0.5 * e * n, 0: 0.0, 1: 0.5 * e * n}    # c2 slot, qy

    cth, sth = math.cos(math.pi / 8), math.sin(math.pi / 8)
    comps = [
        (cth, -sth),               # p1
        (sth, cth),                # p2
    ]

    CH = H // R
    pos_chunk = {0: 0, 1: min(1, CH - 1), 2: CH - 1}
    # stationaries [18, 124, 128]: x slots rows 0..59, c2 slots rows 64..123
    # (rows 60..63 are zero padding -- DVE writes must start at partition
    # 0/32/64/96, so the c2 block sits at 64); index ((ci*3 + pos)*3 + o)
    wst = np.zeros((18, 124, 128), np.float64)
    for ci, (wx, wy) in enumerate(comps):
        for pos in range(3):
            c0 = pos_chunk[pos]
            start = min(max(R * c0 - 2, 0), H - KROWS)
            for o in range(3):                 # column shift o-1 in {-1,0,1}
                Wm = wst[(ci * 3 + pos) * 3 + o]
                cx = wx * axx[o - 1]
                cxc = wx * axc[o - 1]
                cy = wy * ayx[o - 1]
                cyc = wy * ayc[o - 1]
                for o8 in range(8):
                    for r in range(R):
                        y = R * c0 + r
                        col = R * o8 + r
                        for k in range(-2, 3):
                            v = y + k
                            if v < 0:
                                v = -v
                            if v > H - 1:
                                v = 2 * (H - 1) - v
                            rp = v - start
                            assert 0 <= rp < KROWS
                            for cc in range(3):
                                wxa = Ax[o8, cc] * VX[k + 2]
                                wya = Ay[o8, cc] * VY[k + 2]
                                Wm[cc * KROWS + rp, col] += cx * wxa + cy * wya
                                Wm[64 + cc * KROWS + rp, col] += cxc * wxa + cyc * wya

    # pass-2 stationaries [5, 128, 80]: col = 5*j + plane  (j = row in chunk)
    wp = (w_merge[0].astype(np.float64)) * 0.5
    lam = {
        0: np.array([0.0, 0.0, 1.0, 0.0]),    # g   -> C2
        1: np.array([0.0, 1.0, -1.0, 0.0]),   # gA  -> C1 - C2
        2: np.array([0.0, 0.0, -1.0, 1.0]),   # gB  -> C3 - C2
        3: np.array([1.0, -1.0, 1.0, -1.0]),  # gAB -> C0 - C1 + C2 - C3
    }
    wc = np.zeros((5, 128, 80), np.float64)
    for T in range(4):
        for o8 in range(8):
            for j in range(R):
                for k in range(4):
                    wc[T, R * o8 + j, 5 * j + k] = wp[o8] * lam[T][k]
    for o8 in range(8):
        for j in range(R):
            wc[4, R * o8 + j, 5 * j + 4] = 1.0

    lt = low_t.reshape(-1).astype(np.float64)
    ht = high_t.reshape(-1).astype(np.float64)
    l2v = np.zeros((128, 1), np.float32)
    h2v = np.zeros((128, 1), np.float32)
    for o8 in range(8):
        l2v[R * o8:R * (o8 + 1), 0] = (lt[o8] * lt[o8]) if lt[o8] >= 0 else NEG_BIG
        h2v[R * o8:R * (o8 + 1), 0] = (ht[o8] * ht[o8]) if ht[o8] >= 0 else NEG_BIG

    return {
        # stored pre-transposed to the SBUF tile layouts for contiguous DMAs
        "wst": np.ascontiguousarray(wst.transpose(1, 0, 2)).astype(ml_dtypes.bfloat16),
        "wc": np.ascontiguousarray(wc.transpose(1, 0, 2)).astype(ml_dtypes.bfloat16),
        "l2v": l2v,
        "h2v": h2v,
    }


def chunk_start(c, H):
    return min(max(R * c - 2, 0), H - KROWS)


def chunk_pos(c, H):
    CH = H // R
    return 0 if c == 0 else (2 if c == CH - 1 else 1)


def build_program(H, W, n_cores, debug=False):
    """Emit the single-core Tile program (SPMD across cores)."""
    Wp = W + 5               # padded x columns: x[-3 .. W+1]
    CH = H // R
    NS = H // (R * SCH)
    PB = 128
    HB = H // PB
    CPB = PB // R            # chunks per block
    assert H % (R * SCH) == 0 and PB % R == 0 and (SCH * 2) * R == PB

    nc = bacc.Bacc("TRN2", target_bir_lowering=False, debug=False,
                   enable_asserts=True, num_devices=n_cores)

    # x pre-windowed on host: [NS, 60, SCH, Wp] flattened (+8 slack so the
    # +1-shifted load stays in bounds) -> ONE fully-contiguous DMA per
    # stripe for xcm and one for xc2.
    XWN = NS * 60 * SCH * Wp
    xin_t = nc.dram_tensor("xin", (XWN + 8,), BF16, kind="ExternalInput")
    wst_t = nc.dram_tensor("wst", (124, 18, 128), BF16, kind="ExternalInput")
    wc_t = nc.dram_tensor("wc", (128, 5, 80), BF16, kind="ExternalInput")
    l2v_t = nc.dram_tensor("l2v", (128, 1), F32, kind="ExternalInput")
    h2v_t = nc.dram_tensor("h2v", (128, 1), F32, kind="ExternalInput")
    out_t = nc.dram_tensor("out", (H, W), F32, kind="ExternalOutput")
    dbg_t = None
    if debug:
        dbg_t = nc.dram_tensor("dbg", (H, W), F32, kind="ExternalOutput")

    def dram_ap(t, offset, pairs):
        return bass.AP(t, offset, [list(p) for p in pairs])

    from contextlib import ExitStack
    with tile.TileContext(nc) as tc, ExitStack() as stack:
        cpool = stack.enter_context(tc.tile_pool(name="consts", bufs=1))
        wstt = cpool.tile([124, 18, 128], BF16, tag="wst", name="wst")
        wst = [wstt[:, i, :] for i in range(18)]
        wct = cpool.tile([128, 5, 80], BF16, tag="wc", name="wc")
        wc = [wct[:, i, :] for i in range(5)]
        l2v = cpool.tile([128, 1], F32, tag="l2v", name="l2v")
        h2v = cpool.tile([128, 1], F32, tag="h2v", name="h2v")

        def load_consts():
            # emitted after stripe 0's x loads so those win SDMA bandwidth
            nc.scalar.dma_start(wstt[:], wst_t.ap())
            nc.scalar.dma_start(wct[:], wc_t.ap())
            nc.scalar.dma_start(l2v[:], l2v_t.ap())
            nc.scalar.dma_start(h2v[:], h2v_t.ap())
        epsv = cpool.tile([128, 1], F32, tag="epsv", name="epsv")
        nc.gpsimd.memset(epsv[:], float(EPS))

        # full-image planes: P5 holds [C0..C3, s] interleaved per block.
        ppool = stack.enter_context(tc.tile_pool(name="planes", bufs=1))
        P5 = ppool.tile([PB, 5, HB, W], BF16, tag="P5", name="P5")
        spl = ppool.tile([PB, HB, W + 2], BF16, tag="spl", name="spl")
        sSp = ppool.tile([PB, HB, W + 2], BF16, tag="sSp", name="sSp")
        sNp = ppool.tile([PB, HB, W + 2], BF16, tag="sNp", name="sNp")
        mpl = ppool.tile([PB, HB, W], F32, tag="mpl", name="mpl")
        hhpl = ppool.tile([PB, HB, W], BF16, tag="hhpl", name="hhpl")
        iScpl = ppool.tile([PB, HB, W], BF16, tag="iScpl", name="iScpl")

        # zero-fill once: pad cols + out-of-image halos stay zero (the dirs
        # conv uses zero padding); block DMAs overwrite the interiors.
        for t in (spl, sSp, sNp):
            nc.gpsimd.memset(t[:], 0.0)

        with (
            tc.tile_pool(name="xs", bufs=2) as xpool,
            tc.tile_pool(name="sq", bufs=3) as spool,
            tc.tile_pool(name="dv", bufs=2) as dpool,
            tc.tile_pool(name="s2", bufs=3) as s2pool,
            tc.tile_pool(name="tl", bufs=1) as tpool,
            tc.tile_pool(name="ps", bufs=2, space="PSUM") as mm1pool,
            tc.tile_pool(name="ps2", bufs=2, space="PSUM") as mm2pool,
        ):
            stripe_state = {}

            def emit_A(s):
                """Loads + c2 + PE pass 1 + squares/signs."""
                cbase = s * SCH
                xcm = xpool.tile([124, SCH, Wp], BF16, tag="xcm", name="xcm")
                xc2 = xpool.tile([60, SCH, Wp], BF16, tag="xc2", name="xc2")
                # pad rows 60..63: any finite data (stationary rows are zero)
                nc.gpsimd.dma_start(
                    xcm[60:64, :, :],
                    dram_ap(xin_t, 0, [(Wp, 4), (Wp, SCH), (1, Wp)]))
                base = s * (60 * SCH * Wp)
                nc.sync.dma_start(
                    xcm[0:60, :, :],
                    dram_ap(xin_t, base, [(SCH * Wp, 60), (Wp, SCH), (1, Wp)]))
                nc.sync.dma_start(
                    xc2[:, :, :],
                    dram_ap(xin_t, base + 1, [(SCH * Wp, 60), (Wp, SCH), (1, Wp)]))
                # c2 slots: c2[t] = x[t-4] + x[t-2] (both from the +1-shifted copy)
                nc.vector.tensor_tensor(
                    xcm[64:124, :, 2:Wp - 1], xc2[:, :, 0:Wp - 3],
                    xc2[:, :, 2:Wp - 1], OP.add)

                # PE pass 1 (p1, p2): one PSUM tile per chunk; Square + Sign ACTs
                SPall = spool.tile([128, SCH, 2, W], BF16, tag="spall", name="spall")
                SGall = spool.tile([128, SCH, 2, W], BF16, tag="sgall", name="sgall")
                for cl in range(SCH):
                    pos = chunk_pos(cbase + cl, H)
                    pm = mm1pool.tile([128, 2, W], F32, tag="pm", name="pm")
                    for ci in range(2):
                        for o in range(3):
                            nc.tensor.matmul(
                                pm[:, ci, :], wst[(ci * 3 + pos) * 3 + o],
                                xcm[0:124, cl, 2 + o:2 + o + W],
                                start=(o == 0), stop=(o == 2))
                    nc.scalar.activation(SPall[:, cl, :, :], pm[:], AF.Square)
                    nc.scalar.activation(SGall[:, cl, :, :], pm[:], AF.Sign)
                stripe_state[s] = {"SPall": SPall, "SGall": SGall}

            def emit_B(s, h):
                """DVE chain + sqrt for half-stripe h (chunks 2h, 2h+1)."""
                st = stripe_state[s]
                sl = slice(2 * h, 2 * h + 2)
                sp1 = st["SPall"][:, sl, 0, :]
                sp2 = st["SPall"][:, sl, 1, :]
                mag2 = dpool.tile([128, 2, W], BF16, tag="mag2", name="mag2")
                nc.vector.tensor_tensor(mag2[:], sp1, sp2, OP.add)
                bm = dpool.tile([128, 2, W], BF16, tag="bm", name="bm")
                nc.vector.tensor_tensor(bm[:], sp1, sp2, OP.is_ge)
                # alpha = [p1*p2 >= 0] == [sign(p1) == sign(p2)] (ties measure-zero)
                am = dpool.tile([128, 2, W], BF16, tag="am", name="am")
                nc.vector.tensor_tensor(am[:], st["SGall"][:, sl, 0, :],
                                        st["SGall"][:, sl, 1, :], OP.is_equal)
                stepl = dpool.tile([128, 2, W], BF16, tag="stepl", name="stepl")
                nc.vector.tensor_scalar(stepl[:], mag2[:], l2v[:], None, OP.is_ge)
                steph = dpool.tile([128, 2, W], BF16, tag="steph", name="steph")
                nc.vector.tensor_scalar(steph[:], mag2[:], h2v[:], None, OP.is_ge)
                g = dpool.tile([128, 2, W], BF16, tag="g", name="g")
                nc.vector.tensor_tensor(g[:], stepl[:], steph[:], OP.add)
                gA = dpool.tile([128, 2, W], BF16, tag="gA", name="gA")
                nc.vector.tensor_tensor(gA[:], am[:], g[:], OP.mult)
                gB = dpool.tile([128, 2, W], BF16, tag="gB", name="gB")
                nc.vector.tensor_tensor(gB[:], bm[:], g[:], OP.mult)
                gAB = dpool.tile([128, 2, W], BF16, tag="gAB", name="gAB")
                nc.vector.tensor_tensor(gAB[:], am[:], gB[:], OP.mult)
                mag = dpool.tile([128, 2, W], BF16, tag="mag", name="mag")
                nc.scalar.activation(mag[:], mag2[:], AF.Sqrt, bias=epsv[:])
                st[("half", h)] = (g, gA, gB, gAB, mag)

            def emit_C(s, h):
                """PE pass 2 + PSUM copy + repack for half-stripe h."""
                st = stripe_state[s]
                g, gA, gB, gAB, mag = st.pop(("half", h))
                gT = (g, gA, gB, gAB)
                pc = mm2pool.tile([80, 2, W], F32, tag="pc", name="pc")
                for cl2h in range(2):
                    for T in range(4):
                        nc.tensor.matmul(pc[:, cl2h, :], wc[T], gT[T][:, cl2h, :],
                                         start=(T == 0), stop=False)
                    nc.tensor.matmul(pc[:, cl2h, :], wc[4], mag[:, cl2h, :],
                                     start=False, stop=True)
                stC = s2pool.tile([80, 2, W], BF16, tag="stC", name="stC")
                nc.scalar.activation(stC[:], pc[:], AF.Copy)
                for cl2h in range(2):
                    c = s * SCH + 2 * h + cl2h
                    b = c // CPB
                    cl2 = c % CPB
                    nc.gpsimd.dma_start(P5[R * cl2:R * (cl2 + 1), :, b, :],
                                        stC[:, cl2h, :])

            def emit_nms_planes(b, p0=0, p1=PB):
                """Shifted s-plane DMAs for block b (interior + top halo).
                All on HWDGE rings: gpsimd SWDGE descriptor generation is
                port-locked out while DVE runs 2-port ops."""
                nc.gpsimd.dma_start(spl[p0:p1, b, 1:W + 1], P5[p0:p1, 4, b, :])
                # sSp[p] = s[row-1] (N side), sNp[p] = s[row+1] (S side)
                lo = max(p0, 1)
                nc.sync.dma_start(sSp[lo:p1, b, 1:W + 1], P5[lo - 1:p1 - 1, 4, b, :])
                if p0 == 0 and b > 0:
                    nc.sync.dma_start(sSp[0:1, b, 1:W + 1], P5[PB - 1:PB, 4, b - 1, :])
                hi = min(p1, PB - 1)
                nc.gpsimd.dma_start(sNp[p0:hi, b, 1:W + 1], P5[p0 + 1:hi + 1, 4, b, :])

            def emit_nms(b, p0=0, p1=PB, planes_done=False):
                """NMS gating for block b, partition rows [p0, p1)."""
                n = p1 - p0
                sc = P5[p0:p1, 4, b, :]
                if not planes_done:
                    emit_nms_planes(b, p0, p1)
                if p1 == PB and b < HB - 1:
                    nc.sync.dma_start(sNp[PB - 1:PB, b, 1:W + 1], P5[0:1, 4, b + 1, :])

                # axis planes: C0=E/W, C1=SE/NW, C2=S/N, C3=SW/NE
                pairs = [
                    (spl[p0:p1, b, 2:W + 2], spl[p0:p1, b, 0:W]),      # E, W
                    (sNp[p0:p1, b, 2:W + 2], sSp[p0:p1, b, 0:W]),      # SE, NW
                    (sSp[p0:p1, b, 1:W + 1], sNp[p0:p1, b, 1:W + 1]),  # N, S
                    (sNp[p0:p1, b, 0:W], sSp[p0:p1, b, 2:W + 2]),      # SW, NE
                ]
                pks = []
                for k, (va, vb) in enumerate(pairs):
                    mx = tpool.tile([PB, W], BF16, tag="mx", name="mx")
                    nc.vector.tensor_tensor(mx[p0:p1, :], va, vb, OP.max)
                    bk = tpool.tile([PB, W], BF16, tag="bk", name="bk")
                    nc.vector.tensor_tensor(bk[p0:p1, :], sc, mx[p0:p1, :], OP.is_gt)
                    pk = tpool.tile([PB, W], BF16, tag=f"pk{k}", name=f"pk{k}")
                    nc.vector.tensor_tensor(pk[p0:p1, :], bk[p0:p1, :],
                                            P5[p0:p1, k, b, :], OP.mult)
                    pks.append(pk)
                a01 = tpool.tile([PB, W], F32, tag="a01", name="a01")
                nc.vector.tensor_tensor(a01[p0:p1, :], pks[0][p0:p1, :],
                                        pks[1][p0:p1, :], OP.add)
                a23 = tpool.tile([PB, W], F32, tag="a23", name="a23")
                nc.vector.tensor_tensor(a23[p0:p1, :], pks[2][p0:p1, :],
                                        pks[3][p0:p1, :], OP.add)
                nc.vector.tensor_tensor(mpl[p0:p1, b, :], a01[p0:p1, :],
                                        a23[p0:p1, :], OP.add)

                # strong map + horizontal part of the hysteresis box sum
                iSp = tpool.tile([PB, W + 2], BF16, tag="iSp", name="iSp")
                nc.vector.tensor_scalar(iSp[p0:p1, 1:W + 1], mpl[p0:p1, b, :],
                                        1.0, None, OP.is_equal)
                nc.vector.tensor_scalar(iScpl[p0:p1, b, :], mpl[p0:p1, b, :],
                                        1.0, None, OP.is_equal)
                # reflect pad: iS[-1] = iS[1], iS[W] = iS[W-2]
                nc.vector.tensor_copy(iSp[p0:p1, 0:1], iSp[p0:p1, 2:3])
                nc.vector.tensor_copy(iSp[p0:p1, W + 1:W + 2], iSp[p0:p1, W - 1:W])
                th = tpool.tile([PB, W], BF16, tag="th", name="th")
                nc.vector.tensor_tensor(th[p0:p1, :], iSp[p0:p1, 0:W],
                                        iSp[p0:p1, 2:W + 2], OP.add)
                nc.vector.tensor_tensor(hhpl[p0:p1, b, :], th[p0:p1, :],
                                        iScpl[p0:p1, b, :], OP.add)
                if dbg_t is not None:
                    nc.sync.dma_start(
                        dram_ap(dbg_t, b * PB * W + p0 * W, [(W, n), (1, W)]),
                        mpl[p0:p1, b, :])

            def emit_hyst(b):
                hhN = tpool.tile([PB, W], BF16, tag="hhN", name="hhN")
                hhS = tpool.tile([PB, W], BF16, tag="hhS", name="hhS")
                nc.sync.dma_start(hhN[0:PB - 1, :], hhpl[1:PB, b, :])
                nc.sync.dma_start(hhS[1:PB, :], hhpl[0:PB - 1, b, :])
                if b < HB - 1:
                    nc.sync.dma_start(hhN[PB - 1:PB, :], hhpl[0:1, b + 1, :])
                else:  # reflect at y = H-1
                    nc.sync.dma_start(hhN[PB - 1:PB, :], hhpl[PB - 2:PB - 1, b, :])
                if b > 0:
                    nc.sync.dma_start(hhS[0:1, :], hhpl[PB - 1:PB, b - 1, :])
                else:  # reflect at y = 0
                    nc.sync.dma_start(hhS[0:1, :], hhpl[1:2, b, :])
                vv = tpool.tile([PB, W], BF16, tag="vv", name="vv")
                nc.vector.tensor_tensor(vv[:], hhN[:], hhS[:], OP.add)
                hv = tpool.tile([PB, W], BF16, tag="hv", name="hv")
                nc.vector.tensor_tensor(hv[:], vv[:], hhpl[:, b, :], OP.add)
                # w2 = 0.5*[hv > 0.8]; w1 = w2*[m == 0.5]; out = w1 + iS
                w2 = tpool.tile([PB, W], BF16, tag="w2", name="w2")
                nc.vector.tensor_scalar(w2[:], hv[:], 0.8, 0.5, OP.is_gt, OP.mult)
                m5 = tpool.tile([PB, W], BF16, tag="m5", name="m5")
                nc.vector.tensor_scalar(m5[:], mpl[:, b, :], 0.5, None, OP.is_equal)
                w1 = tpool.tile([PB, W], BF16, tag="w1", name="w1")
                nc.vector.tensor_tensor(w1[:], w2[:], m5[:], OP.mult)
                outb = tpool.tile([PB, W], BF16, tag="outb", name="outb")
                nc.vector.tensor_tensor(outb[:], w1[:], iScpl[:, b, :], OP.add)
                outv = tpool.tile([PB, W], F32, tag="outv", name="outv")
                nc.scalar.activation(outv[:], outb[:], AF.Copy)
                nc.sync.dma_start(
                    dram_ap(out_t, b * PB * W, [(W, PB), (1, W)]), outv[:])

            emit_A(0)
            load_consts()
            for s in range(NS):
                if s + 1 < NS:
                    emit_A(s + 1)
                last = (s == NS - 1)
                for h in range(2):
                    emit_B(s, h)
                    emit_C(s, h)
                    if last:
                        # last block split by partition halves so the
                        # endgame latency chain is short
                        emit_nms(HB - 1, 64 * h, 64 * (h + 1))
                        if h == 0:
                            emit_hyst(HB - 2)
                stripe_state.pop(s)
                if not last and s % 2 == 1:
                    # interior shifted-plane DMAs can start one stripe early
                    emit_nms_planes((s - 1) // 2)
                if not last and s >= 2 and s % 2 == 0:
                    emit_nms(s // 2 - 1, planes_done=True)
                if not last and s >= 4 and s % 2 == 0:
                    emit_hyst(s // 2 - 2)
            emit_hyst(HB - 1)

    nc.compile()
    return nc


_PROG_CACHE = {}


def _get_program(H, W, n_cores):
    key = (H, W, n_cores, DEBUG)
    if key not in _PROG_CACHE:
        _PROG_CACHE[key] = build_program(H, W, n_cores, debug=DEBUG)
    return _PROG_CACHE[key]


def make_in_maps(x, w_expand, w_perm_gauss, w_perm_sx, w_perm_sy, w_merge,
                 low_t, high_t):
    B, Cin, H, W = x.shape
    Wp = W + 5
    NS = H // (R * SCH)
    wd = make_weights(np.asarray(w_expand, np.float64), np.asarray(w_perm_gauss, np.float64),
                      np.asarray(w_perm_sx, np.float64), np.asarray(w_perm_sy, np.float64),
                      np.asarray(w_merge, np.float64), np.asarray(low_t), np.asarray(high_t), H)
    xpad = np.pad(np.asarray(x, np.float32), ((0, 0), (0, 0), (0, 0), (3, 2)),
                  mode="reflect").astype(ml_dtypes.bfloat16)
    # pre-windowed layout [NS, 60 = 3ch x KROWS, SCH, Wp] (+8 slack elements)
    xwin = np.zeros((B, NS, 3, KROWS, SCH, Wp), ml_dtypes.bfloat16)
    for s in range(NS):
        for cl in range(SCH):
            st = chunk_start(s * SCH + cl, H)
            xwin[:, s, :, :, cl, :] = xpad[:, :, st:st + KROWS, :]
    xflat = np.zeros((B, NS * 60 * SCH * Wp + 8), ml_dtypes.bfloat16)
    xflat[:, :-8] = xwin.reshape(B, -1)
    shared = {
        "wst": np.ascontiguousarray(wd["wst"]),
        "wc": np.ascontiguousarray(wd["wc"]),
        "l2v": wd["l2v"], "h2v": wd["h2v"],
    }
    return [dict(shared, xin=xflat[b]) for b in range(B)]


def kernel(x, w_expand, w_perm_gauss, w_perm_sx, w_perm_sy, w_merge,
           low_t, high_t):
    x = np.asarray(x)
    B, Cin, H, W = x.shape
    assert (B, Cin) == (8, 3)
    nc = _get_program(H, W, 8)
    in_maps = make_in_maps(x, w_expand, w_perm_gauss, w_perm_sx, w_perm_sy,
                           w_merge, low_t, high_t)
    res = run_bass_kernel_spmd(nc, in_maps, core_ids=list(range(8)))
    out = np.stack([res.results[b]["out"] for b in range(8)])[:, None]
    return out.astype(np.float32)


# revision 47
# speedup vs baseline: 1.1085x; 1.0501x over previous
"""Trainium2 Bass kernel for nn_CannyDetector (B=8, Cin=3, C=8, H=W=512).

Strategy (pure data parallel, one image per NeuronCore, 8 cores):

Reference pipeline per core:
    h  = Wexp @ x; g = Pg @ gauss3x3(h); gx = Px @ sobelx(g); gy = Py @ sobely(g)
    mag = sqrt(gx^2+gy^2+eps); s = sum_c mag; dirs_k = s - shift_k(s)
    idx = round(atan2(gy,gx)/45deg); nms = mag * (dirs[idx]>0 & dirs[idx+4]>0)
    thr = step functions of mag; m = Wmerge @ thr; out = hysteresis(m)

This implementation folds ALL linear work (gauss/sobel separable 5x5
stencils + channel mixes + the +22.5deg rotation) into TensorE:
  - one DVE op per stripe builds the c2 = x[w-1]+x[w+1] helper slots; the
    moving operand {x, c2} (120 partitions = 2 slots x 3ch x 20 rows) then
    needs only 3 column-shifted accumulating matmuls per component.
  - components are p1,p2 (gradients rotated by +22.5deg).  The NMS
    sector tests reduce to sign tests: beta = [p1^2>=p2^2],
    alpha = [2*q1^2>=mag2] == [p1*p2>=0] == [sign(p1)==sign(p2)].
  - ScalarE emits squares and signs of the PE outputs (PSUM->SBUF bf16);
    the remaining per-channel chain is 7 TT + 2 TS DVE ops per
    half-stripe, software-pipelined so PE pass-1(s+1) overlaps chain(s).
  - PE pass 2 contracts channels into 4 axis-plane tensors C_k and
    s = sum_c mag with an interleaved column layout (col = 5*row + plane)
    so ONE SBUF->SBUF DMA per chunk repacks all 5 planes into the
    full-image row-major tail layout.
  - the 1-channel tail (directional NMS gating + hysteresis) runs
    per 128-row block, pipelined into the stripe loop, so only the last
    block's work trails the final stripe.
"""

import math
import sys

import numpy as np

if "/opt/trn_rl_repo" not in sys.path:
    sys.path.insert(0, "/opt/trn_rl_repo")

import ml_dtypes

import concourse.bass as bass
import concourse.bacc as bacc
import concourse.tile as tile
from concourse import mybir
from concourse.bass_utils import run_bass_kernel_spmd

F32 = mybir.dt.float32
BF16 = mybir.dt.bfloat16
OP = mybir.AluOpType
AF = mybir.ActivationFunctionType

EPS = 1e-10
NEG_BIG = -1.0e30

R = 16           # rows per chunk (PE pass output = 8ch x 16 rows)
KROWS = R + 4    # vertical halo rows per chunk (5-tap)
SCH = 4          # chunks per stripe
DEBUG = False    # adds a dbg m-plane output when building the program


def make_weights(w_expand, w_perm_gauss, w_perm_sx, w_perm_sy, w_merge,
                 low_t, high_t, H):
    """Host-side prep of stationary matrices and per-partition vectors."""
    e = math.exp(-1.0 / 18.0)
    n = 1.0 / (2.0 * e + 1.0)

    Ax = (w_perm_sx @ w_perm_gauss @ w_expand).astype(np.float64)   # (8,3)
    Ay = (w_perm_sy @ w_perm_gauss @ w_expand).astype(np.float64)

    gv = np.array([e, 1.0, e]) * n
    VX = np.convolve(np.array([0.5, 1.0, 0.5]), gv)   # qx vertical 5-tap
    VY = np.convolve(np.array([-1.0, 0.0, 1.0]), gv)  # qy vertical 5-tap

    # horizontal FIRs decomposed over moving slots {x, c2=x[w-1]+x[w+1]}
    # qx path: H = conv([-1,0,1], [e,1,e])*n ; qy path: H = conv([.5,1,.5],[e,1,e])*n
    axx = {-1: -n, 0: 0.0, 1: n}            # x slot, qx
    axc = {-1: -e * n, 0: 0.0, 1: e * n}    # c2 slot, qx
    ayx = {-1: (0.5 + e) * n, 0: n, 1: (0.5 + e) * n}  # x slot, qy
    ayc = {-1: 0.5 * e * n, 0: 0.0, 1: 0.5 * e * n}    # c2 slot, qy

    cth, sth = math.cos(math.pi / 8), math.sin(math.pi / 8)
    comps = [
        (cth, -sth),               # p1
        (sth, cth),                # p2
    ]

    CH = H // R
    pos_chunk = {0: 0, 1: min(1, CH - 1), 2: CH - 1}
    # stationaries [18, 124, 128]: x slots rows 0..59, c2 slots rows 64..123
    # (rows 60..63 are zero padding -- DVE writes must start at partition
    # 0/32/64/96, so the c2 block sits at 64); index ((ci*3 + pos)*3 + o)
    wst = np.zeros((18, 124, 128), np.float64)
    for ci, (wx, wy) in enumerate(comps):
        for pos in range(3):
            c0 = pos_chunk[pos]
            start = min(max(R * c0 - 2, 0), H - KROWS)
            for o in range(3):                 # column shift o-1 in {-1,0,1}
                Wm = wst[(ci * 3 + pos) * 3 + o]
                cx = wx * axx[o - 1]
                cxc = wx * axc[o - 1]
                cy = wy * ayx[o - 1]
                cyc = wy * ayc[o - 1]
                for o8 in range(8):
                    for r in range(R):
                        y = R * c0 + r
                        col = R * o8 + r
                        for k in range(-2, 3):
                            v = y + k
                            if v < 0:
                                v = -v
                            if v > H - 1:
                                v = 2 * (H - 1) - v
                            rp = v - start
                            assert 0 <= rp < KROWS
                            for cc in range(3):
                                wxa = Ax[o8, cc] * VX[k + 2]
                                wya = Ay[o8, cc] * VY[k + 2]
                                Wm[cc * KROWS + rp, col] += cx * wxa + cy * wya
                                Wm[64 + cc * KROWS + rp, col] += cxc * wxa + cyc * wya

    # pass-2 stationaries [5, 128, 80]: col = 5*j + plane  (j = row in chunk)
    wp = (w_merge[0].astype(np.float64)) * 0.5
    lam = {
        0: np.array([0.0, 0.0, 1.0, 0.0]),    # g   -> C2
        1: np.array([0.0, 1.0, -1.0, 0.0]),   # gA  -> C1 - C2
        2: np.array([0.0, 0.0, -1.0, 1.0]),   # gB  -> C3 - C2
        3: np.array([1.0, -1.0, 1.0, -1.0]),  # gAB -> C0 - C1 + C2 - C3
    }
    wc = np.zeros((5, 128, 80), np.float64)
    for T in range(4):
        for o8 in range(8):
            for j in range(R):
                for k in range(4):
                    wc[T, R * o8 + j, 5 * j + k] = wp[o8] * lam[T][k]
    for o8 in range(8):
        for j in range(R):
            wc[4, R * o8 + j, 5 * j + 4] = 1.0

    lt = low_t.reshape(-1).astype(np.float64)
    ht = high_t.reshape(-1).astype(np.float64)
    l2v = np.zeros((128, 1), np.float32)
    h2v = np.zeros((128, 1), np.float32)
    for o8 in range(8):
        l2v[R * o8:R * (o8 + 1), 0] = (lt[o8] * lt[o8]) if lt[o8] >= 0 else NEG_BIG
        h2v[R * o8:R * (o8 + 1), 0] = (ht[o8] * ht[o8]) if ht[o8] >= 0 else NEG_BIG

    return {
        # stored pre-transposed to the SBUF tile layouts for contiguous DMAs
        "wst": np.ascontiguousarray(wst.transpose(1, 0, 2)).astype(ml_dtypes.bfloat16),
        "wc": np.ascontiguousarray(wc.transpose(1, 0, 2)).astype(ml_dtypes.bfloat16),
        "l2v": l2v,
        "h2v": h2v,
    }


def chunk_start(c, H):
    return min(max(R * c - 2, 0), H - KROWS)


def chunk_pos(c, H):
    CH = H // R
    return 0 if c == 0 else (2 if c == CH - 1 else 1)


def build_program(H, W, n_cores, debug=False):
    """Emit the single-core Tile program (SPMD across cores)."""
    Wp = W + 5               # padded x columns: x[-3 .. W+1]
    CH = H // R
    NS = H // (R * SCH)
    PB = 128
    HB = H // PB
    CPB = PB // R            # chunks per block
    assert H % (R * SCH) == 0 and PB % R == 0 and (SCH * 2) * R == PB

    nc = bacc.Bacc("TRN2", target_bir_lowering=False, debug=False,
                   enable_asserts=True, num_devices=n_cores)

    # x pre-windowed on host: [NS, 60, SCH, Wp] flattened (+8 slack so the
    # +1-shifted load stays in bounds) -> ONE fully-contiguous DMA per
    # stripe for xcm and one for xc2.
    XWN = NS * 60 * SCH * Wp
    xin_t = nc.dram_tensor("xin", (XWN + 8,), BF16, kind="ExternalInput")
    wst_t = nc.dram_tensor("wst", (124, 18, 128), BF16, kind="ExternalInput")
    wc_t = nc.dram_tensor("wc", (128, 5, 80), BF16, kind="ExternalInput")
    l2v_t = nc.dram_tensor("l2v", (128, 1), F32, kind="ExternalInput")
    h2v_t = nc.dram_tensor("h2v", (128, 1), F32, kind="ExternalInput")
    out_t = nc.dram_tensor("out", (H, W), F32, kind="ExternalOutput")
    dbg_t = None
    if debug:
        dbg_t = nc.dram_tensor("dbg", (H, W), F32, kind="ExternalOutput")

    def dram_ap(t, offset, pairs):
        return bass.AP(t, offset, [list(p) for p in pairs])

    from contextlib import ExitStack
    with tile.TileContext(nc) as tc, ExitStack() as stack:
        cpool = stack.enter_context(tc.tile_pool(name="consts", bufs=1))
        wstt = cpool.tile([124, 18, 128], BF16, tag="wst", name="wst")
        nc.scalar.dma_start(wstt[:], wst_t.ap())
        wst = [wstt[:, i, :] for i in range(18)]
        wct = cpool.tile([128, 5, 80], BF16, tag="wc", name="wc")
        nc.scalar.dma_start(wct[:], wc_t.ap())
        wc = [wct[:, i, :] for i in range(5)]
        l2v = cpool.tile([128, 1], F32, tag="l2v", name="l2v")
        nc.scalar.dma_start(l2v[:], l2v_t.ap())
        h2v = cpool.tile([128, 1], F32, tag="h2v", name="h2v")
        nc.scalar.dma_start(h2v[:], h2v_t.ap())
        epsv = cpool.tile([128, 1], F32, tag="epsv", name="epsv")
        nc.gpsimd.memset(epsv[:], float(EPS))

        # full-image planes: P5 holds [C0..C3, s] interleaved per block.
        ppool = stack.enter_context(tc.tile_pool(name="planes", bufs=1))
        P5 = ppool.tile([PB, 5, HB, W], BF16, tag="P5", name="P5")
        spl = ppool.tile([PB, HB, W + 2], BF16, tag="spl", name="spl")
        sSp = ppool.tile([PB, HB, W + 2], BF16, tag="sSp", name="sSp")
        sNp = ppool.tile([PB, HB, W + 2], BF16, tag="sNp", name="sNp")
        mpl = ppool.tile([PB, HB, W], F32, tag="mpl", name="mpl")
        hhpl = ppool.tile([PB, HB, W], BF16, tag="hhpl", name="hhpl")
        iScpl = ppool.tile([PB, HB, W], BF16, tag="iScpl", name="iScpl")

        # zero-fill once: pad cols + out-of-image halos stay zero (the dirs
        # conv uses zero padding); block DMAs overwrite the interiors.
        for t in (spl, sSp, sNp):
            nc.gpsimd.memset(t[:], 0.0)

        with (
            tc.tile_pool(name="xs", bufs=2) as xpool,
            tc.tile_pool(name="sq", bufs=3) as spool,
            tc.tile_pool(name="dv", bufs=2) as dpool,
            tc.tile_pool(name="s2", bufs=3) as s2pool,
            tc.tile_pool(name="tl", bufs=1) as tpool,
            tc.tile_pool(name="ps", bufs=2, space="PSUM") as mm1pool,
            tc.tile_pool(name="ps2", bufs=2, space="PSUM") as mm2pool,
        ):
            stripe_state = {}

            def emit_A(s):
                """Loads + c2 + PE pass 1 + squares/signs."""
                cbase = s * SCH
                xcm = xpool.tile([124, SCH, Wp], BF16, tag="xcm", name="xcm")
                xc2 = xpool.tile([60, SCH, Wp], BF16, tag="xc2", name="xc2")
                # pad rows 60..63: any finite data (stationary rows are zero)
                nc.gpsimd.dma_start(
                    xcm[60:64, :, :],
                    dram_ap(xin_t, 0, [(Wp, 4), (Wp, SCH), (1, Wp)]))
                base = s * (60 * SCH * Wp)
                nc.sync.dma_start(
                    xcm[0:60, :, :],
                    dram_ap(xin_t, base, [(SCH * Wp, 60), (Wp, SCH), (1, Wp)]))
                nc.sync.dma_start(
                    xc2[:, :, :],
                    dram_ap(xin_t, base + 1, [(SCH * Wp, 60), (Wp, SCH), (1, Wp)]))
                # c2 slots: c2[t] = x[t-4] + x[t-2] (both from the +1-shifted copy)
                nc.vector.tensor_tensor(
                    xcm[64:124, :, 2:Wp - 1], xc2[:, :, 0:Wp - 3],
                    xc2[:, :, 2:Wp - 1], OP.add)

                # PE pass 1 (p1, p2): one PSUM tile per chunk; Square + Sign ACTs
                SPall = spool.tile([128, SCH, 2, W], BF16, tag="spall", name="spall")
                SGall = spool.tile([128, SCH, 2, W], BF16, tag="sgall", name="sgall")
                for cl in range(SCH):
                    pos = chunk_pos(cbase + cl, H)
                    pm = mm1pool.tile([128, 2, W], F32, tag="pm", name="pm")
                    for ci in range(2):
                        for o in range(3):
                            nc.tensor.matmul(
                                pm[:, ci, :], wst[(ci * 3 + pos) * 3 + o],
                                xcm[0:124, cl, 2 + o:2 + o + W],
                                start=(o == 0), stop=(o == 2))
                    nc.scalar.activation(SPall[:, cl, :, :], pm[:], AF.Square)
                    nc.scalar.activation(SGall[:, cl, :, :], pm[:], AF.Sign)
                stripe_state[s] = {"SPall": SPall, "SGall": SGall}

            def emit_B(s, h):
                """DVE chain + sqrt for half-stripe h (chunks 2h, 2h+1)."""
                st = stripe_state[s]
                sl = slice(2 * h, 2 * h + 2)
                sp1 = st["SPall"][:, sl, 0, :]
                sp2 = st["SPall"][:, sl, 1, :]
                mag2 = dpool.tile([128, 2, W], BF16, tag="mag2", name="mag2")
                nc.vector.tensor_tensor(mag2[:], sp1, sp2, OP.add)
                bm = dpool.tile([128, 2, W], BF16, tag="bm", name="bm")
                nc.vector.tensor_tensor(bm[:], sp1, sp2, OP.is_ge)
                # alpha = [p1*p2 >= 0] == [sign(p1) == sign(p2)] (ties measure-zero)
                am = dpool.tile([128, 2, W], BF16, tag="am", name="am")
                nc.vector.tensor_tensor(am[:], st["SGall"][:, sl, 0, :],
                                        st["SGall"][:, sl, 1, :], OP.is_equal)
                stepl = dpool.tile([128, 2, W], BF16, tag="stepl", name="stepl")
                nc.vector.tensor_scalar(stepl[:], mag2[:], l2v[:], None, OP.is_ge)
                steph = dpool.tile([128, 2, W], BF16, tag="steph", name="steph")
                nc.vector.tensor_scalar(steph[:], mag2[:], h2v[:], None, OP.is_ge)
                g = dpool.tile([128, 2, W], BF16, tag="g", name="g")
                nc.vector.tensor_tensor(g[:], stepl[:], steph[:], OP.add)
                gA = dpool.tile([128, 2, W], BF16, tag="gA", name="gA")
                nc.vector.tensor_tensor(gA[:], am[:], g[:], OP.mult)
                gB = dpool.tile([128, 2, W], BF16, tag="gB", name="gB")
                nc.vector.tensor_tensor(gB[:], bm[:], g[:], OP.mult)
                gAB = dpool.tile([128, 2, W], BF16, tag="gAB", name="gAB")
                nc.vector.tensor_tensor(gAB[:], am[:], gB[:], OP.mult)
                mag = dpool.tile([128, 2, W], BF16, tag="mag", name="mag")
                nc.scalar.activation(mag[:], mag2[:], AF.Sqrt, bias=epsv[:])
                st[("half", h)] = (g, gA, gB, gAB, mag)

            def emit_C(s, h):
                """PE pass 2 + PSUM copy + repack for half-stripe h."""
                st = stripe_state[s]
                g, gA, gB, gAB, mag = st.pop(("half", h))
                gT = (g, gA, gB, gAB)
                pc = mm2pool.tile([80, 2, W], F32, tag="pc", name="pc")
                for cl2h in range(2):
                    for T in range(4):
                        nc.tensor.matmul(pc[:, cl2h, :], wc[T], gT[T][:, cl2h, :],
                                         start=(T == 0), stop=False)
                    nc.tensor.matmul(pc[:, cl2h, :], wc[4], mag[:, cl2h, :],
                                     start=False, stop=True)
                stC = s2pool.tile([80, 2, W], BF16, tag="stC", name="stC")
                nc.scalar.activation(stC[:], pc[:], AF.Copy)
                for cl2h in range(2):
                    c = s * SCH + 2 * h + cl2h
                    b = c // CPB
                    cl2 = c % CPB
                    nc.gpsimd.dma_start(P5[R * cl2:R * (cl2 + 1), :, b, :],
                                        stC[:, cl2h, :])

            def emit_nms_planes(b, p0=0, p1=PB):
                """Shifted s-plane DMAs for block b (interior + top halo)."""
                nc.gpsimd.dma_start(spl[p0:p1, b, 1:W + 1], P5[p0:p1, 4, b, :])
                # sSp[p] = s[row-1] (N side), sNp[p] = s[row+1] (S side)
                lo = max(p0, 1)
                nc.sync.dma_start(sSp[lo:p1, b, 1:W + 1], P5[lo - 1:p1 - 1, 4, b, :])
                if p0 == 0 and b > 0:
                    nc.sync.dma_start(sSp[0:1, b, 1:W + 1], P5[PB - 1:PB, 4, b - 1, :])
                hi = min(p1, PB - 1)
                nc.gpsimd.dma_start(sNp[p0:hi, b, 1:W + 1], P5[p0 + 1:hi + 1, 4, b, :])

            def emit_nms(b, p0=0, p1=PB, planes_done=False):
                """NMS gating for block b, partition rows [p0, p1)."""
                n = p1 - p0
                sc = P5[p0:p1, 4, b, :]
                if not planes_done:
                    emit_nms_planes(b, p0, p1)
                if p1 == PB and b < HB - 1:
                    nc.gpsimd.dma_start(sNp[PB - 1:PB, b, 1:W + 1], P5[0:1, 4, b + 1, :])

                # axis planes: C0=E/W, C1=SE/NW, C2=S/N, C3=SW/NE
                pairs = [
                    (spl[p0:p1, b, 2:W + 2], spl[p0:p1, b, 0:W]),      # E, W
                    (sNp[p0:p1, b, 2:W + 2], sSp[p0:p1, b, 0:W]),      # SE, NW
                    (sSp[p0:p1, b, 1:W + 1], sNp[p0:p1, b, 1:W + 1]),  # N, S
                    (sNp[p0:p1, b, 0:W], sSp[p0:p1, b, 2:W + 2]),      # SW, NE
                ]
                pks = []
                for k, (va, vb) in enumerate(pairs):
                    mx = tpool.tile([PB, W], BF16, tag="mx", name="mx")
                    nc.vector.tensor_tensor(mx[p0:p1, :], va, vb, OP.max)
                    bk = tpool.tile([PB, W], BF16, tag="bk", name="bk")
                    nc.vector.tensor_tensor(bk[p0:p1, :], sc, mx[p0:p1, :], OP.is_gt)
                    pk = tpool.tile([PB, W], BF16, tag=f"pk{k}", name=f"pk{k}")
                    nc.vector.tensor_tensor(pk[p0:p1, :], bk[p0:p1, :],
                                            P5[p0:p1, k, b, :], OP.mult)
                    pks.append(pk)
                a01 = tpool.tile([PB, W], F32, tag="a01", name="a01")
                nc.vector.tensor_tensor(a01[p0:p1, :], pks[0][p0:p1, :],
                                        pks[1][p0:p1, :], OP.add)
                a23 = tpool.tile([PB, W], F32, tag="a23", name="a23")
                nc.vector.tensor_tensor(a23[p0:p1, :], pks[2][p0:p1, :],
                                        pks[3][p0:p1, :], OP.add)
                nc.vector.tensor_tensor(mpl[p0:p1, b, :], a01[p0:p1, :],
                                        a23[p0:p1, :], OP.add)

                # strong map + horizontal part of the hysteresis box sum
                iSp = tpool.tile([PB, W + 2], BF16, tag="iSp", name="iSp")
                nc.vector.tensor_scalar(iSp[p0:p1, 1:W + 1], mpl[p0:p1, b, :],
                                        1.0, None, OP.is_equal)
                nc.vector.tensor_scalar(iScpl[p0:p1, b, :], mpl[p0:p1, b, :],
                                        1.0, None, OP.is_equal)
                # reflect pad: iS[-1] = iS[1], iS[W] = iS[W-2]
                nc.vector.tensor_copy(iSp[p0:p1, 0:1], iSp[p0:p1, 2:3])
                nc.vector.tensor_copy(iSp[p0:p1, W + 1:W + 2], iSp[p0:p1, W - 1:W])
                th = tpool.tile([PB, W], BF16, tag="th", name="th")
                nc.vector.tensor_tensor(th[p0:p1, :], iSp[p0:p1, 0:W],
                                        iSp[p0:p1, 2:W + 2], OP.add)
                nc.vector.tensor_tensor(hhpl[p0:p1, b, :], th[p0:p1, :],
                                        iScpl[p0:p1, b, :], OP.add)
                if dbg_t is not None:
                    nc.sync.dma_start(
                        dram_ap(dbg_t, b * PB * W + p0 * W, [(W, n), (1, W)]),
                        mpl[p0:p1, b, :])

            def emit_hyst(b):
                hhN = tpool.tile([PB, W], BF16, tag="hhN", name="hhN")
                hhS = tpool.tile([PB, W], BF16, tag="hhS", name="hhS")
                nc.sync.dma_start(hhN[0:PB - 1, :], hhpl[1:PB, b, :])
                nc.gpsimd.dma_start(hhS[1:PB, :], hhpl[0:PB - 1, b, :])
                if b < HB - 1:
                    nc.sync.dma_start(hhN[PB - 1:PB, :], hhpl[0:1, b + 1, :])
                else:  # reflect at y = H-1
                    nc.sync.dma_start(hhN[PB - 1:PB, :], hhpl[PB - 2:PB - 1, b, :])
                if b > 0:
                    nc.gpsimd.dma_start(hhS[0:1, :], hhpl[PB - 1:PB, b - 1, :])
                else:  # reflect at y = 0
                    nc.gpsimd.dma_start(hhS[0:1, :], hhpl[1:2, b, :])
                vv = tpool.tile([PB, W], BF16, tag="vv", name="vv")
                nc.vector.tensor_tensor(vv[:], hhN[:], hhS[:], OP.add)
                hv = tpool.tile([PB, W], BF16, tag="hv", name="hv")
                nc.vector.tensor_tensor(hv[:], vv[:], hhpl[:, b, :], OP.add)
                # w2 = 0.5*[hv > 0.8]; w1 = w2*[m == 0.5]; out = w1 + iS
                w2 = tpool.tile([PB, W], BF16, tag="w2", name="w2")
                nc.vector.tensor_scalar(w2[:], hv[:], 0.8, 0.5, OP.is_gt, OP.mult)
                m5 = tpool.tile([PB, W], BF16, tag="m5", name="m5")
                nc.vector.tensor_scalar(m5[:], mpl[:, b, :], 0.5, None, OP.is_equal)
                w1 = tpool.tile([PB, W], BF16, tag="w1", name="w1")
                nc.vector.tensor_tensor(w1[:], w2[:], m5[:], OP.mult)
                outb = tpool.tile([PB, W], BF16, tag="outb", name="outb")
                nc.vector.tensor_tensor(outb[:], w1[:], iScpl[:, b, :], OP.add)
                outv = tpool.tile([PB, W], F32, tag="outv", name="outv")
                nc.scalar.activation(outv[:], outb[:], AF.Copy)
                nc.sync.dma_start(
                    dram_ap(out_t, b * PB * W, [(W, PB), (1, W)]), outv[:])

            emit_A(0)
            for s in range(NS):
                if s + 1 < NS:
                    emit_A(s + 1)
                last = (s == NS - 1)
                for h in range(2):
                    emit_B(s, h)
                    emit_C(s, h)
                    if last:
                        # last block split by partition halves so the
                        # endgame latency chain is short
                        emit_nms(HB - 1, 64 * h, 64 * (h + 1))
                        if h == 0:
                            emit_hyst(HB - 2)
                stripe_state.pop(s)
                if not last and s % 2 == 1:
                    # interior shifted-plane DMAs can start one stripe early
                    emit_nms_planes((s - 1) // 2)
                if not last and s >= 2 and s % 2 == 0:
                    emit_nms(s // 2 - 1, planes_done=True)
                if not last and s >= 4 and s % 2 == 0:
                    emit_hyst(s // 2 - 2)
            emit_hyst(HB - 1)

    nc.compile()
    return nc


_PROG_CACHE = {}


def _get_program(H, W, n_cores):
    key = (H, W, n_cores, DEBUG)
    if key not in _PROG_CACHE:
        _PROG_CACHE[key] = build_program(H, W, n_cores, debug=DEBUG)
    return _PROG_CACHE[key]


def make_in_maps(x, w_expand, w_perm_gauss, w_perm_sx, w_perm_sy, w_merge,
                 low_t, high_t):
    B, Cin, H, W = x.shape
    Wp = W + 5
    NS = H // (R * SCH)
    wd = make_weights(np.asarray(w_expand, np.float64), np.asarray(w_perm_gauss, np.float64),
                      np.asarray(w_perm_sx, np.float64), np.asarray(w_perm_sy, np.float64),
                      np.asarray(w_merge, np.float64), np.asarray(low_t), np.asarray(high_t), H)
    xpad = np.pad(np.asarray(x, np.float32), ((0, 0), (0, 0), (0, 0), (3, 2)),
                  mode="reflect").astype(ml_dtypes.bfloat16)
    # pre-windowed layout [NS, 60 = 3ch x KROWS, SCH, Wp] (+8 slack elements)
    xwin = np.zeros((B, NS, 3, KROWS, SCH, Wp), ml_dtypes.bfloat16)
    for s in range(NS):
        for cl in range(SCH):
            st = chunk_start(s * SCH + cl, H)
            xwin[:, s, :, :, cl, :] = xpad[:, :, st:st + KROWS, :]
    xflat = np.zeros((B, NS * 60 * SCH * Wp + 8), ml_dtypes.bfloat16)
    xflat[:, :-8] = xwin.reshape(B, -1)
    shared = {
        "wst": np.ascontiguousarray(wd["wst"]),
        "wc": np.ascontiguousarray(wd["wc"]),
        "l2v": wd["l2v"], "h2v": wd["h2v"],
    }
    return [dict(shared, xin=xflat[b]) for b in range(B)]


def kernel(x, w_expand, w_perm_gauss, w_perm_sx, w_perm_sy, w_merge,
           low_t, high_t):
    x = np.asarray(x)
    B, Cin, H, W = x.shape
    assert (B, Cin) == (8, 3)
    nc = _get_program(H, W, 8)
    in_maps = make_in_maps(x, w_expand, w_perm_gauss, w_perm_sx, w_perm_sy,
                           w_merge, low_t, high_t)
    res = run_bass_kernel_spmd(nc, in_maps, core_ids=list(range(8)))
    out = np.stack([res.results[b]["out"] for b in range(8)])[:, None]
    return out.astype(np.float32)
